# revision 52
# baseline (speedup 1.0000x reference)
"""Sparse (relu-cosine, causal+padding-masked) attention on 8 TRN2 NeuronCores.

Contract: kernel(**inputs) takes the full unsharded inputs and returns the
full [B, S, D] output. Internally:
  - host: compact each batch's tokens to the mask-valid ones (queries and
    keys share the same validity mask, so causal structure stays exactly
    lower-triangular in compacted space and all masking disappears),
    transpose X, slice per-head-pair weights, pad to tile multiples.
  - device (SPMD, 8 cores, 2 heads per core): QKV projections, cosine
    normalization folded into the relu scale (1/||k||) and a per-query
    broadcast tile (1/||q||), relu(QK^T) with triangular masks only on
    diagonal tiles, context accumulation (col-tiled pairs), and a partial
    output projection (transposed layout) through this core's 128 columns
    of Wo.
  - host: sum the 8 partial outputs, scatter rows back to the full
    [B, S, D] layout (masked query rows are exactly zero).

Matmul operands are bf16; every accumulation (PSUM) is fp32 and the
norm scales (1/||q||, 1/||k||) are computed from the fp32 sums, so the
cosine normalization is exact for the bf16-rounded Q/K. Attention is
software-pipelined per q-block: scores of block i+1 are issued to the PE
before the context matmuls of block i, so the PE never waits on relu.
"""

import numpy as np

B, S, D, H = 2, 2048, 1024, 16
DH = D // H
NCORES = 8
HEADS_PER_CORE = H // NCORES  # 2
NH = HEADS_PER_CORE
JW = HEADS_PER_CORE * DH  # 128, per-core head-dim slice width
QB = 512  # query block width (one fp32 PSUM bank)
KT = 128  # key tile (partition dim)


def _build_program(LQs, n_dblk=D // 128):
    import concourse.bass as bass
    import concourse.mybir as mybir
    import concourse.tile as tile
    from concourse import bacc
    from concourse.bass import ts
    from concourse.masks import make_identity

    F32 = mybir.dt.float32
    F32R = mybir.dt.float32r
    F16 = mybir.dt.float16
    BF16 = mybir.dt.bfloat16
    F8E4 = mybir.dt.float8e4
    DROW = mybir.MatmulPerfMode.DoubleRow
    AF = mybir.ActivationFunctionType
    MULT = mybir.AluOpType.mult
    MAX = mybir.AluOpType.max

    LT = sum(LQs)
    offs = [0, LQs[0]]
    n_ttiles = LT // 128
    NBLK = -(-LT // QB)
    LT_pad = NBLK * QB

    nc = bacc.Bacc("TRN2", target_bir_lowering=False, debug=False,
                   num_devices=NCORES)

    # all big DRAM tensors are host-prearranged so every DMA is one
    # instruction with large per-partition-contiguous descriptors (one
    # dma_start already spreads over all 16 SDMA engines; many small
    # dma_starts just pay ~0.7us DIRECT2D issue cost each on the sync
    # sequencer).
    XT = nc.dram_tensor("XT", [NBLK, 128, n_dblk * QB], BF16,
                        kind="ExternalInput").ap()
    # q/k/v weights batched into one DMA: [128, 3, n_dblk*JW]
    WALL = nc.dram_tensor("WALL", [128, 3 * n_dblk * JW], BF16,
                          kind="ExternalInput").ap()
    WOT = nc.dram_tensor("WOT", [JW, D], BF16, kind="ExternalInput").ap()
    # causal diag-tile masks: CAUS[:, 384-off : 384-off+qbw], off = kt0-q0
    CAUS = nc.dram_tensor("CAUS", [128, 896], BF16, kind="ExternalInput").ap()
    # IND[j, h] = 1 if j // DH == h ; INDT is its transpose
    IND = nc.dram_tensor("IND", [JW, NH], BF16, kind="ExternalInput").ap()
    INDT = nc.dram_tensor("INDT", [NH, JW], F32R, kind="ExternalInput").ap()
    # transposed output, per-q-block contiguous; host reassembles
    n_qblocks = sum(-(-lq // QB) for lq in LQs)
    OUTT = nc.dram_tensor("OUTT", [n_qblocks, 128, n_dblk * QB], F16,
                          kind="ExternalOutput").ap()

    EPS = 1e-12

    def col_blocks(width, bw=QB):
        blocks = []
        c = 0
        while c < width:
            w = min(bw, width - c)
            blocks.append((c, w))
            c += w
        return blocks

    with tile.TileContext(nc) as tc:
        with (
            tc.tile_pool(name="consts", bufs=1) as consts,
            tc.tile_pool(name="proj", bufs=1) as projp,
            tc.tile_pool(name="work", bufs=3) as work,
            tc.tile_pool(name="outp", bufs=2) as outp,
            tc.tile_pool(name="ps_mm", bufs=1, space="PSUM") as ps_mm,
            tc.tile_pool(name="ps_scp", bufs=3, space="PSUM") as ps_scp,
            tc.tile_pool(name="ps_ctxp", bufs=1, space="PSUM") as ps_ctxp,
        ):
            # ---- weights first (first projection matmul needs them); the
            # sync sequencer issues [weights, xt blocks] only, in that order:
            # every DIRECT2D issue ahead of the first xt block delays the PE
            # ramp, so everything else goes through the scalar HWDGE queue.
            wall = consts.tile([128, 3, n_dblk, JW], BF16)
            nc.sync.dma_start(
                out=wall,
                in_=WALL.rearrange("p (s k j) -> p s k j", s=3, k=n_dblk))
            wqt, wkt, wvt = wall[:, 0], wall[:, 1], wall[:, 2]

            xtp = tc.alloc_tile_pool(name="xt", bufs=1)
            # ---- X^T (all d-blocks resident), one DMA per 512-col block ----
            # block-major layout: the first projection block lands first, and
            # each DMA is 1MB of 8KB-contiguous per-partition runs.
            xt = xtp.tile([128, NBLK, n_dblk, QB], BF16)
            for b in range(NBLK):
                nc.sync.dma_start(
                    out=xt[:, b],
                    in_=XT[b].rearrange("p (k w) -> p k w", k=n_dblk))

            # ---- remaining constants (scalar HWDGE queue) ------------------
            caus = consts.tile([128, 896], BF16)
            nc.scalar.dma_start(out=caus, in_=CAUS[:, :])
            wot = consts.tile([JW, D], BF16)
            nc.scalar.dma_start(out=wot, in_=WOT[:, :])
            ind = consts.tile([JW, NH], BF16)
            nc.scalar.dma_start(out=ind, in_=IND[:, :])
            indt = consts.tile([NH, JW], F32R)
            nc.scalar.dma_start(out=indt, in_=INDT[:, :])
            eps128 = consts.tile([128, 1], F32)
            nc.vector.memset(eps128, EPS)
            ident = consts.tile([128, 128], BF16)
            make_identity(nc, ident)

            # ---- projections ------------------------------------------------
            # qt/kt/vt in bf16; squares for the norms are taken from the f32
            # PSUM so the scales stay exact for the rounded Q/K.
            qt = projp.tile([JW, LT], BF16)
            kt_ = projp.tile([JW, LT], BF16)
            vt = projp.tile([JW, LT], BF16)
            qsq = projp.tile([JW, LT], BF16)
            ksq = projp.tile([JW, LT], BF16)
            cp_i = 0
            for bi, (c0, w) in enumerate(col_blocks(LT)):
                for dst, wmat, sq in ((qt, wqt, qsq), (kt_, wkt, ksq),
                                      (vt, wvt, None)):
                    ps = ps_scp.tile([JW, QB], F32, tag="sc", name="ps_proj")
                    for k in range(n_dblk):
                        nc.tensor.matmul(
                            ps[:, :w], wmat[:, k, :], xt[:, bi, k, :w],
                            start=(k == 0), stop=(k == n_dblk - 1),
                        )
                    if cp_i % 2 == 0:
                        nc.vector.tensor_copy(dst[:, c0:c0 + w], ps[:, :w])
                    else:
                        nc.scalar.activation(out=dst[:, c0:c0 + w],
                                             in_=ps[:, :w], func=AF.Copy)
                    cp_i += 1
                    if sq is not None:
                        # gpsimd is otherwise idle and these are SBUF->SBUF
                        nc.gpsimd.tensor_mul(sq[:, c0:c0 + w],
                                             dst[:, c0:c0 + w],
                                             dst[:, c0:c0 + w])

            # ---- kscale[t, h] = rsqrt(sum_j ksq[j, t] over head h) ----------
            ksum_ps = ps_ctxp.tile([128, n_ttiles * NH], F32, tag="ctx_ps",
                                   name="ksum_ps")
            for tt in range(n_ttiles):
                nc.tensor.matmul(ksum_ps[:, tt * NH:(tt + 1) * NH],
                                 ksq[:, ts(tt, 128)], ind[:, :],
                                 start=True, stop=True, skip_group_check=True)
            ksc = projp.tile([128, n_ttiles, NH], F32)
            nc.scalar.activation(out=ksc[:, :, :].rearrange("p a b -> p (a b)"),
                                 in_=ksum_ps[:, :], func=AF.Sqrt,
                                 bias=eps128[:, :], scale=1.0)
            nc.vector.reciprocal_approx_fast(
                out=ksc[:, :, :].rearrange("p a b -> p (a b)"),
                in_=ksc[:, :, :].rearrange("p a b -> p (a b)"))

            # ---- V natural via PE transpose --------------------------------
            vn = projp.tile([128, n_ttiles, JW], BF16)
            for tt in range(n_ttiles):
                ps = ps_scp.tile([128, 128], BF16, tag="sc", name="ps_vtr")
                nc.tensor.transpose(ps[:, :], vt[:, ts(tt, 128)], ident)
                for h in range(NH):
                    if tt % 2 == 0:
                        nc.vector.tensor_scalar_mul(
                            out=vn[:, tt, ts(h, DH)], in0=ps[:, ts(h, DH)],
                            scalar1=ksc[:, tt, h:h + 1])
                    else:
                        nc.scalar.activation(
                            out=vn[:, tt, ts(h, DH)], in_=ps[:, ts(h, DH)],
                            func=AF.Copy, scale=ksc[:, tt, h:h + 1])

            # keep xt resident: releasing it here would make the att pool
            # reuse its SBUF zone, serializing attention start behind the
            # last projection matmul. Both fit in SBUF at bf16 sizes.
            max_nkt = max(LQs) // KT
            att_bufs = 3 if max_nkt <= 10 else (2 if max_nkt <= 14 else 1)
            attp = tc.alloc_tile_pool(name="att", bufs=att_bufs)



            # ---- attention, software-pipelined over q-blocks ----------------
            blocks = []
            for b in range(B):
                for q0, qw in col_blocks(LQs[b]):
                    blocks.append((b, q0, qw))
            ctx_sbs = {
                b: attp.tile([JW, LQs[b]], BF16, tag=f"ctx_{b}", bufs=1,
                             name=f"ctx_sb{b}")
                for b in range(B)
            }

            state = {}

            def emit_scores(blk):
                b, q0, qw = blk
                ob = offs[b]
                lq = LQs[b]
                # QSB[p, q] = rsqrt(|q|^2 + eps)[q, head(p)] broadcast tile
                ps_ss = ps_mm.tile([NH, QB], F32, tag="mm", name="ps_qsum")
                nc.tensor.matmul(ps_ss[:, :qw], ind[:, :],
                                 qsq[:, ob + q0:ob + q0 + qw],
                                 start=True, stop=True)
                ssq = work.tile([NH, QB], F32R, tag="ssq")
                nc.scalar.activation(out=ssq[:, :qw], in_=ps_ss[:, :qw],
                                     func=AF.Sqrt, bias=eps128[:NH, :],
                                     scale=1.0)
                # broadcast |q| to all 128 partitions first, then take the
                # reciprocal on the full tile: same serial length on the DVE
                # (free-dim bound) but it replaces the PSUM->SBUF copy, and
                # reciprocal_approx_fast is ~5x faster than InstReciprocal.
                ps_qsb = ps_mm.tile([128, QB], F32, tag="mm", name="ps_qsb")
                nc.tensor.matmul(ps_qsb[:, :qw], indt[:, :], ssq[:, :qw],
                                 start=True, stop=True)
                qsb = work.tile([128, QB], F32, tag="qsb")
                nc.vector.reciprocal_approx_fast(out=qsb[:, :qw],
                                                 in_=ps_qsb[:, :qw])

                n_kt = min((q0 + qw + KT - 1) // KT, lq // KT)
                att_sb = attp.tile([128, max_nkt * NH, QB], BF16,
                                   tag="att_sb", name="att_sb")
                offs_ki = []
                diag_i = 0
                relu_i = 0
                for ki in range(n_kt):
                    k0 = ki * KT
                    # columns < off are fully masked by causality; skip them
                    off = max(0, k0 - q0)
                    offs_ki.append(off)
                    w = qw - off
                    diag = k0 > q0 - KT
                    sc_ps = ps_scp.tile([128, NH, QB], F32, tag="sc",
                                        name="sc_ps")
                    for h in range(NH):
                        nc.tensor.matmul(
                            sc_ps[:, h, off:qw],
                            kt_[ts(h, DH), ob + k0:ob + k0 + KT],
                            qt[ts(h, DH), ob + q0 + off:ob + q0 + qw],
                            start=True, stop=True,
                        )
                    # att = relu(s) for both heads in one op (k-norm scale
                    # lives in V); diagonal tiles fuse the triangular mask:
                    # (s max 0) * caus, with caus broadcast over the head dim
                    sl = ki * NH
                    slot = att_sb[:, sl:sl + NH, off:qw]
                    if diag:
                        cs = caus[:, 384:384 + w]
                        cs2 = bass.AP(tensor=cs.tensor, offset=cs.offset,
                                      ap=[cs.ap[0], [0, NH], cs.ap[1]])
                        if diag_i % 2 == 0:
                            nc.vector.scalar_tensor_tensor(
                                out=slot, in0=sc_ps[:, :, off:qw], scalar=0.0,
                                in1=cs2, op0=MAX, op1=MULT)
                        else:
                            nc.scalar.activation(
                                out=slot, in_=sc_ps[:, :, off:qw],
                                func=AF.Relu)
                            nc.vector.tensor_mul(slot, slot, cs2)
                        diag_i += 1
                    else:
                        # DVE max is ~1.5x faster per element than the ACT
                        # relu here; scalar gets the out-proj copies instead
                        if relu_i % 4 == 3:
                            nc.scalar.activation(
                                out=slot, in_=sc_ps[:, :, off:qw],
                                func=AF.Relu)
                        else:
                            nc.vector.tensor_scalar_max(
                                out=slot, in0=sc_ps[:, :, off:qw],
                                scalar1=0.0)
                        relu_i += 1
                state[blk] = (att_sb, qsb, n_kt, offs_ki)

            def emit_ctx_out(blk_i, blk):
                b, q0, qw = blk
                ob = offs[b]
                ctx_sb = ctx_sbs[b]
                att_sb, qsb, n_kt, offs_ki = state.pop(blk)
                # col-tiled pair: both heads accumulate in one PSUM bank
                ctx_ps = ps_ctxp.tile([128, QB], F32, tag="ctx_ps",
                                      name="ctx_ps")
                assert offs_ki[0] == 0  # first tile always starts the bank
                for ki in range(n_kt):
                    gtt = (ob + ki * KT) // KT
                    off = offs_ki[ki]
                    for h in range(NH):
                        nc.tensor.matmul(
                            ctx_ps[ts(h, DH), off:qw],
                            vn[:, gtt, ts(h, DH)],
                            att_sb[:, ki * NH + h, off:qw],
                            start=(ki == 0), stop=(ki == n_kt - 1),
                            tile_position=(0, h * DH),
                            skip_group_check=True,
                        )
                # apply 1/|q| while copying ctx out of PSUM
                nc.vector.tensor_mul(ctx_sb[:, q0:q0 + qw], ctx_ps[:, :qw],
                                     qsb[:, :qw])

                # output projection (transposed layout), this q-block only;
                # dblk pairs share one 2-bank PSUM tile; all 8 dout chunks
                # gather into one SBUF tile and ship in a single DMA.
                o_all = outp.tile([128, n_dblk, QB], F16, tag="o_sb")
                dst = OUTT[blk_i].rearrange("p (g w) -> p g w", g=n_dblk)
                last = blk_i == n_qblocks - 1
                for dp in range(n_dblk // 2):
                    ps = ps_scp.tile([128, 2, QB], F32, tag="sc",
                                     name="ps_out")
                    for two in range(2):
                        nc.tensor.matmul(ps[:, two, :qw],
                                         wot[:, ts(dp * 2 + two, 128)],
                                         ctx_sb[:, q0:q0 + qw],
                                         start=True, stop=True)
                    nc.scalar.activation(
                        out=o_all[:, dp * 2:dp * 2 + 2, :qw],
                        in_=ps[:, :, :qw], func=AF.Copy)
                    if last and dp == n_dblk // 2 - 2:
                        # overlap most of the final store with the last
                        # chunk's compute to shorten the drain tail
                        nc.sync.dma_start(out=dst[:, :dp * 2 + 2, :qw],
                                          in_=o_all[:, :dp * 2 + 2, :qw])
                if last:
                    g0 = n_dblk - 2
                    nc.sync.dma_start(out=dst[:, g0:, :qw],
                                      in_=o_all[:, g0:, :qw])
                else:
                    nc.sync.dma_start(out=dst[:, :, :qw],
                                      in_=o_all[:, :, :qw])

            for i, blk in enumerate(blocks):
                emit_scores(blk)
                if i > 0:
                    emit_ctx_out(i - 1, blocks[i - 1])
            emit_ctx_out(len(blocks) - 1, blocks[-1])
            attp.release()
            xtp.release()

    nc.compile()
    return nc


def _prepare(X, masks, Wq, Wk, Wv, Wo):
    import ml_dtypes
    BF = ml_dtypes.bfloat16
    F8 = ml_dtypes.float8_e4m3

    X = np.asarray(X, dtype=np.float32)
    masks = np.asarray(masks)
    Wq = np.asarray(Wq, dtype=np.float32)
    Wk = np.asarray(Wk, dtype=np.float32)
    Wv = np.asarray(Wv, dtype=np.float32)
    Wo = np.asarray(Wo, dtype=np.float32)

    idxs = [np.where(masks[b] != 0)[0] for b in range(B)]
    # 256-multiples so fp8 DoubleRow k-tile pairs align for both batches
    LQs = [max(256, int(-(-len(ix) // 256) * 256)) for ix in idxs]
    LT = sum(LQs)
    offs = [0, LQs[0]]
    QBK = 512
    NBLK = -(-LT // QBK)
    LT_pad = NBLK * QBK
    n_dblk = D // 128

    # compacted, transposed X: columns = valid tokens (zero-padded)
    XTc = np.zeros((D, LT_pad), dtype=np.float32)
    for b in range(B):
        XTc[:, offs[b]:offs[b] + len(idxs[b])] = X[b].T[:, idxs[b]]
    # DMA-friendly: [NBLK, 128, n_dblk*QBK], per-partition contiguous
    XTa = np.ascontiguousarray(
        XTc.reshape(n_dblk, 128, NBLK, QBK).transpose(2, 1, 0, 3)
        .reshape(NBLK, 128, n_dblk * QBK)).astype(BF)

    caus = (np.arange(896)[None, :] - 384 >= np.arange(128)[:, None])

    nc = _build_program(LQs)

    def warr(wT):  # [D, JW] -> [128, n_dblk*JW] per-partition contiguous
        return np.ascontiguousarray(
            wT.reshape(n_dblk, 128, JW).transpose(1, 0, 2)
            .reshape(128, n_dblk * JW)).astype(BF)

    in_maps = []
    for c in range(NCORES):
        jsl = slice(c * JW, (c + 1) * JW)
        ind = np.zeros((JW, NH), dtype=np.float32)
        for h in range(NH):
            ind[h * DH:(h + 1) * DH, h] = 1.0
        in_maps.append({
            "XT": XTa,
            "WALL": np.ascontiguousarray(np.concatenate(
                [warr(Wq[jsl, :].T), warr(Wk[jsl, :].T),
                 warr(Wv[jsl, :].T)], axis=1)),
            "WOT": np.ascontiguousarray(Wo[:, jsl].T).astype(BF),
            "CAUS": caus.astype(BF),
            "IND": ind.astype(BF),
            "INDT": np.ascontiguousarray(ind.T),
        })

    return nc, in_maps, (idxs, LQs, LT, offs)


def _unshard(results, meta):
    idxs, LQs, LT, offs = meta
    n_dblk = D // 128
    blocks = []
    for b in range(B):
        q0 = 0
        while q0 < LQs[b]:
            qw = min(QB, LQs[b] - q0)
            blocks.append((b, q0, qw))
            q0 += qw

    partial = np.zeros((D, LT), dtype=np.float64)
    for c in range(NCORES):
        # OUTT[i, p, g*QB + w] = out[g*128 + p, ob + q0 + w] for block i
        ot = results[c]["OUTT"].astype(np.float64).reshape(
            len(blocks), 128, n_dblk, QB)
        for i, (b, q0, qw) in enumerate(blocks):
            cols = slice(offs[b] + q0, offs[b] + q0 + qw)
            partial[:, cols] += ot[i, :, :, :qw].transpose(1, 0, 2).reshape(
                D, qw)
    partial = partial.T  # [LT, D]

    out = np.zeros((B, S, D), dtype=np.float32)
    for b in range(B):
        out[b, idxs[b], :] = partial[offs[b]:offs[b] + len(idxs[b]), :].astype(
            np.float32)
    return out


def kernel(X, masks, Wq, Wk, Wv, Wo):
    from concourse.bass_utils import run_bass_kernel_spmd

    nc, in_maps, meta = _prepare(X, masks, Wq, Wk, Wv, Wo)
    res = run_bass_kernel_spmd(nc, in_maps, list(range(NCORES)))
    return _unshard(res.results, meta)


def profile_run(inputs, tmpdir=None):
    """Used by test.py: same program, run with NTFF tracing enabled."""
    from concourse.bass_utils import run_bass_kernel_spmd

    nc, in_maps, meta = _prepare(**inputs)
    res = run_bass_kernel_spmd(nc, in_maps, list(range(NCORES)), trace=True,
                               tmpdir=tmpdir)
    res.output = _unshard(res.results, meta)
    return res



# revision 57
# speedup vs baseline: 1.1485x; 1.1485x over previous
"""Sparse (relu-cosine, causal+padding-masked) attention on 8 TRN2 NeuronCores.

Contract: kernel(**inputs) takes the full unsharded inputs and returns the
full [B, S, D] output. Internally:
  - host: compact each batch's tokens to the mask-valid ones (queries and
    keys share the same validity mask, so causal structure stays exactly
    lower-triangular in compacted space and all masking disappears),
    transpose X, slice per-head-pair weights, pad to tile multiples.
  - device (SPMD, 8 cores, 2 heads per core): QKV projections, cosine
    normalization folded into the relu scale (1/||k||) and a per-query
    broadcast tile (1/||q||), relu(QK^T) with triangular masks only on
    diagonal tiles, context accumulation (col-tiled pairs), and a partial
    output projection (transposed layout) through this core's 128 columns
    of Wo.
  - host: sum the 8 partial outputs, scatter rows back to the full
    [B, S, D] layout (masked query rows are exactly zero).

Matmul operands are bf16; every accumulation (PSUM) is fp32 and the
norm scales (1/||q||, 1/||k||) are computed from the fp32 sums, so the
cosine normalization is exact for the bf16-rounded Q/K. Attention is
software-pipelined per q-block: scores of block i+1 are issued to the PE
before the context matmuls of block i, so the PE never waits on relu.
"""

import numpy as np

B, S, D, H = 2, 2048, 1024, 16
DH = D // H
NCORES = 8
HEADS_PER_CORE = H // NCORES  # 2
NH = HEADS_PER_CORE
JW = HEADS_PER_CORE * DH  # 128, per-core head-dim slice width
QB = 512  # query block width (one fp32 PSUM bank)
KT = 128  # key tile (partition dim)


def _build_program(LQs, n_dblk=D // 128):
    import concourse.bass as bass
    import concourse.mybir as mybir
    import concourse.tile as tile
    from concourse import bacc
    from concourse.bass import ts
    from concourse.masks import make_identity

    F32 = mybir.dt.float32
    F32R = mybir.dt.float32r
    F16 = mybir.dt.float16
    BF16 = mybir.dt.bfloat16
    F8E4 = mybir.dt.float8e4
    DROW = mybir.MatmulPerfMode.DoubleRow
    AF = mybir.ActivationFunctionType
    MULT = mybir.AluOpType.mult
    MAX = mybir.AluOpType.max

    LT = sum(LQs)
    offs = [0, LQs[0]]
    n_ttiles = LT // 128
    NBLK = -(-LT // QB)
    LT_pad = NBLK * QB

    nc = bacc.Bacc("TRN2", target_bir_lowering=False, debug=False,
                   num_devices=NCORES)

    # all big DRAM tensors are host-prearranged so every DMA is one
    # instruction with large per-partition-contiguous descriptors (one
    # dma_start already spreads over all 16 SDMA engines; many small
    # dma_starts just pay ~0.7us DIRECT2D issue cost each on the sync
    # sequencer).
    XT = nc.dram_tensor("XT", [NBLK, 128, n_dblk * QB], BF16,
                        kind="ExternalInput").ap()
    # q/k/v weights batched into one DMA: [128, 3, n_dblk*JW]
    WALL = nc.dram_tensor("WALL", [128, 3 * n_dblk * JW], BF16,
                          kind="ExternalInput").ap()
    WOT = nc.dram_tensor("WOT", [JW, D], BF16, kind="ExternalInput").ap()
    # causal diag-tile masks: CAUS[:, 384-off : 384-off+qbw], off = kt0-q0
    CAUS = nc.dram_tensor("CAUS", [128, 896], BF16, kind="ExternalInput").ap()
    # IND[j, h] = 1 if j // DH == h ; INDT is its transpose
    IND = nc.dram_tensor("IND", [JW, NH], BF16, kind="ExternalInput").ap()
    INDT = nc.dram_tensor("INDT", [NH, JW], F32R, kind="ExternalInput").ap()
    # transposed output, per-q-block contiguous; host reassembles
    n_qblocks = sum(-(-lq // QB) for lq in LQs)
    OUTT = nc.dram_tensor("OUTT", [n_qblocks, 128, n_dblk * QB], F16,
                          kind="ExternalOutput").ap()

    EPS = 1e-12

    def col_blocks(width, bw=QB):
        blocks = []
        c = 0
        while c < width:
            w = min(bw, width - c)
            blocks.append((c, w))
            c += w
        return blocks

    with tile.TileContext(nc) as tc:
        with (
            tc.tile_pool(name="consts", bufs=1) as consts,
            tc.tile_pool(name="proj", bufs=1) as projp,
            tc.tile_pool(name="work", bufs=3) as work,
            tc.tile_pool(name="outp", bufs=2) as outp,
            tc.tile_pool(name="ps_mm", bufs=1, space="PSUM") as ps_mm,
            tc.tile_pool(name="ps_scp", bufs=3, space="PSUM") as ps_scp,
            tc.tile_pool(name="ps_ctxp", bufs=1, space="PSUM") as ps_ctxp,
        ):
            # ---- weights first (first projection matmul needs them); the
            # sync sequencer issues [weights, xt blocks] only, in that order:
            # every DIRECT2D issue ahead of the first xt block delays the PE
            # ramp, so everything else goes through the scalar HWDGE queue.
            wall = consts.tile([128, 3, n_dblk, JW], BF16)
            nc.sync.dma_start(
                out=wall,
                in_=WALL.rearrange("p (s k j) -> p s k j", s=3, k=n_dblk))
            wqt, wkt, wvt = wall[:, 0], wall[:, 1], wall[:, 2]

            xtp = tc.alloc_tile_pool(name="xt", bufs=1)
            # ---- X^T (all d-blocks resident), one DMA per 512-col block ----
            # block-major layout: the first projection block lands first, and
            # each DMA is 1MB of 8KB-contiguous per-partition runs.
            xt = xtp.tile([128, NBLK, n_dblk, QB], BF16)
            for b in range(NBLK):
                nc.sync.dma_start(
                    out=xt[:, b],
                    in_=XT[b].rearrange("p (k w) -> p k w", k=n_dblk))

            # ---- remaining constants (scalar HWDGE queue) ------------------
            caus = consts.tile([128, 896], BF16)
            nc.scalar.dma_start(out=caus, in_=CAUS[:, :])
            wot = consts.tile([JW, D], BF16)
            nc.scalar.dma_start(out=wot, in_=WOT[:, :])
            ind = consts.tile([JW, NH], BF16)
            nc.scalar.dma_start(out=ind, in_=IND[:, :])
            indt = consts.tile([NH, JW], F32R)
            nc.scalar.dma_start(out=indt, in_=INDT[:, :])
            eps128 = consts.tile([128, 1], F32)
            nc.vector.memset(eps128, EPS)
            ident = consts.tile([128, 128], BF16)
            make_identity(nc, ident)

            # ---- projections ------------------------------------------------
            # qt/kt/vt in bf16; squares for the norms are taken from the f32
            # PSUM so the scales stay exact for the rounded Q/K.
            qt = projp.tile([JW, LT], BF16)
            kt_ = projp.tile([JW, LT], BF16)
            vt = projp.tile([JW, LT], BF16)
            qsq = projp.tile([JW, LT], BF16)
            ksq = projp.tile([JW, LT], BF16)
            cp_i = 0
            for bi, (c0, w) in enumerate(col_blocks(LT)):
                for dst, wmat, sq in ((qt, wqt, qsq), (kt_, wkt, ksq),
                                      (vt, wvt, None)):
                    ps = ps_scp.tile([JW, QB], F32, tag="sc", name="ps_proj")
                    for k in range(n_dblk):
                        nc.tensor.matmul(
                            ps[:, :w], wmat[:, k, :], xt[:, bi, k, :w],
                            start=(k == 0), stop=(k == n_dblk - 1),
                        )
                    if cp_i % 2 == 0:
                        nc.vector.tensor_copy(dst[:, c0:c0 + w], ps[:, :w])
                    else:
                        nc.scalar.activation(out=dst[:, c0:c0 + w],
                                             in_=ps[:, :w], func=AF.Copy)
                    cp_i += 1
                    if sq is not None:
                        # gpsimd is otherwise idle and these are SBUF->SBUF
                        nc.gpsimd.tensor_mul(sq[:, c0:c0 + w],
                                             dst[:, c0:c0 + w],
                                             dst[:, c0:c0 + w])

            # ---- normalize queries in place: qt *= 1/|q| --------------------
            # (hoisted out of the attention loop; scores are linear in q and
            # qsb partition p carries head(p)'s scale, matching qt's layout)
            for c0, w in col_blocks(LT):
                ps_ss = ps_mm.tile([NH, QB], F32, tag="mm", name="ps_qsum")
                nc.tensor.matmul(ps_ss[:, :w], ind[:, :], qsq[:, c0:c0 + w],
                                 start=True, stop=True)
                ssq = work.tile([NH, QB], F32R, tag="ssq")
                nc.scalar.activation(out=ssq[:, :w], in_=ps_ss[:, :w],
                                     func=AF.Sqrt, bias=eps128[:NH, :],
                                     scale=1.0)
                ps_qsb = ps_mm.tile([128, QB], F32, tag="mm", name="ps_qsb")
                nc.tensor.matmul(ps_qsb[:, :w], indt[:, :], ssq[:, :w],
                                 start=True, stop=True)
                qsb = work.tile([128, QB], F32, tag="qsb")
                nc.vector.reciprocal_approx_fast(out=qsb[:, :w],
                                                 in_=ps_qsb[:, :w])
                nc.vector.tensor_mul(qt[:, c0:c0 + w], qt[:, c0:c0 + w],
                                     qsb[:, :w])

            # ---- kscale[t, h] = rsqrt(sum_j ksq[j, t] over head h) ----------
            ksum_ps = ps_ctxp.tile([128, n_ttiles * NH], F32, tag="ctx_ps",
                                   name="ksum_ps")
            for tt in range(n_ttiles):
                nc.tensor.matmul(ksum_ps[:, tt * NH:(tt + 1) * NH],
                                 ksq[:, ts(tt, 128)], ind[:, :],
                                 start=True, stop=True, skip_group_check=True)
            ksc = projp.tile([128, n_ttiles, NH], F32)
            nc.scalar.activation(out=ksc[:, :, :].rearrange("p a b -> p (a b)"),
                                 in_=ksum_ps[:, :], func=AF.Sqrt,
                                 bias=eps128[:, :], scale=1.0)
            nc.vector.reciprocal_approx_fast(
                out=ksc[:, :, :].rearrange("p a b -> p (a b)"),
                in_=ksc[:, :, :].rearrange("p a b -> p (a b)"))

            # ---- V natural via PE transpose --------------------------------
            vn = projp.tile([128, n_ttiles, JW], BF16)
            for tt in range(n_ttiles):
                ps = ps_scp.tile([128, 128], BF16, tag="sc", name="ps_vtr")
                nc.tensor.transpose(ps[:, :], vt[:, ts(tt, 128)], ident)
                for h in range(NH):
                    if tt % 2 == 0:
                        nc.vector.tensor_scalar_mul(
                            out=vn[:, tt, ts(h, DH)], in0=ps[:, ts(h, DH)],
                            scalar1=ksc[:, tt, h:h + 1])
                    else:
                        nc.scalar.activation(
                            out=vn[:, tt, ts(h, DH)], in_=ps[:, ts(h, DH)],
                            func=AF.Copy, scale=ksc[:, tt, h:h + 1])

            # keep xt resident: releasing it here would make the att pool
            # reuse its SBUF zone, serializing attention start behind the
            # last projection matmul. Both fit in SBUF at bf16 sizes.
            max_nkt = max(LQs) // KT
            att_bufs = 3 if max_nkt <= 10 else (2 if max_nkt <= 14 else 1)
            attp = tc.alloc_tile_pool(name="att", bufs=att_bufs)



            # ---- attention, software-pipelined over q-blocks ----------------
            blocks = []
            for b in range(B):
                for q0, qw in col_blocks(LQs[b]):
                    blocks.append((b, q0, qw))
            ctx_sbs = {
                b: attp.tile([JW, LQs[b]], BF16, tag=f"ctx_{b}", bufs=1,
                             name=f"ctx_sb{b}")
                for b in range(B)
            }

            state = {}

            def emit_scores(blk):
                b, q0, qw = blk
                ob = offs[b]
                lq = LQs[b]
                n_kt = min((q0 + qw + KT - 1) // KT, lq // KT)
                att_sb = attp.tile([128, max_nkt * NH, QB], BF16,
                                   tag="att_sb", name="att_sb")
                offs_ki = []
                diag_i = 0
                relu_i = 0
                for ki in range(n_kt):
                    k0 = ki * KT
                    # columns < off are fully masked by causality; skip them
                    off = max(0, k0 - q0)
                    offs_ki.append(off)
                    w = qw - off
                    diag = k0 > q0 - KT
                    sc_ps = ps_scp.tile([128, NH, QB], F32, tag="sc",
                                        name="sc_ps")
                    for h in range(NH):
                        nc.tensor.matmul(
                            sc_ps[:, h, off:qw],
                            kt_[ts(h, DH), ob + k0:ob + k0 + KT],
                            qt[ts(h, DH), ob + q0 + off:ob + q0 + qw],
                            start=True, stop=True,
                        )
                    # att = relu(s) for both heads in one op (k-norm scale
                    # lives in V); diagonal tiles fuse the triangular mask:
                    # (s max 0) * caus, with caus broadcast over the head dim
                    sl = ki * NH
                    slot = att_sb[:, sl:sl + NH, off:qw]
                    if diag:
                        cs = caus[:, 384:384 + w]
                        cs2 = bass.AP(tensor=cs.tensor, offset=cs.offset,
                                      ap=[cs.ap[0], [0, NH], cs.ap[1]])
                        if diag_i % 2 == 0:
                            nc.vector.scalar_tensor_tensor(
                                out=slot, in0=sc_ps[:, :, off:qw], scalar=0.0,
                                in1=cs2, op0=MAX, op1=MULT)
                        else:
                            nc.scalar.activation(
                                out=slot, in_=sc_ps[:, :, off:qw],
                                func=AF.Relu)
                            nc.vector.tensor_mul(slot, slot, cs2)
                        diag_i += 1
                    else:
                        # DVE max is ~1.5x faster per element than the ACT
                        # relu here; scalar gets the out-proj copies instead
                        if relu_i % 4 == 3:
                            nc.scalar.activation(
                                out=slot, in_=sc_ps[:, :, off:qw],
                                func=AF.Relu)
                        else:
                            nc.vector.tensor_scalar_max(
                                out=slot, in0=sc_ps[:, :, off:qw],
                                scalar1=0.0)
                        relu_i += 1
                state[blk] = (att_sb, n_kt, offs_ki)

            def emit_ctx_out(blk_i, blk):
                b, q0, qw = blk
                ob = offs[b]
                ctx_sb = ctx_sbs[b]
                att_sb, n_kt, offs_ki = state.pop(blk)
                # col-tiled pair: both heads accumulate in one PSUM bank
                ctx_ps = ps_ctxp.tile([128, QB], F32, tag="ctx_ps",
                                      name="ctx_ps")
                assert offs_ki[0] == 0  # first tile always starts the bank
                for ki in range(n_kt):
                    gtt = (ob + ki * KT) // KT
                    off = offs_ki[ki]
                    for h in range(NH):
                        nc.tensor.matmul(
                            ctx_ps[ts(h, DH), off:qw],
                            vn[:, gtt, ts(h, DH)],
                            att_sb[:, ki * NH + h, off:qw],
                            start=(ki == 0), stop=(ki == n_kt - 1),
                            tile_position=(0, h * DH),
                            skip_group_check=True,
                        )
                # 1/|q| already folded into the queries; plain copy, on the
                # scalar engine to keep vector off this block-boundary chain
                nc.scalar.activation(out=ctx_sb[:, q0:q0 + qw],
                                     in_=ctx_ps[:, :qw], func=AF.Copy)

                # output projection (transposed layout), this q-block only;
                # dblk pairs share one 2-bank PSUM tile; all 8 dout chunks
                # gather into one SBUF tile and ship in a single DMA.
                o_all = outp.tile([128, n_dblk, QB], F16, tag="o_sb")
                dst = OUTT[blk_i].rearrange("p (g w) -> p g w", g=n_dblk)
                last = blk_i == n_qblocks - 1
                for dp in range(n_dblk // 2):
                    ps = ps_scp.tile([128, 2, QB], F32, tag="sc",
                                     name="ps_out")
                    for two in range(2):
                        nc.tensor.matmul(ps[:, two, :qw],
                                         wot[:, ts(dp * 2 + two, 128)],
                                         ctx_sb[:, q0:q0 + qw],
                                         start=True, stop=True)
                    nc.scalar.activation(
                        out=o_all[:, dp * 2:dp * 2 + 2, :qw],
                        in_=ps[:, :, :qw], func=AF.Copy)
                    if last and dp == n_dblk // 2 - 2:
                        # overlap most of the final store with the last
                        # chunk's compute to shorten the drain tail
                        nc.sync.dma_start(out=dst[:, :dp * 2 + 2, :qw],
                                          in_=o_all[:, :dp * 2 + 2, :qw])
                if last:
                    g0 = n_dblk - 2
                    nc.sync.dma_start(out=dst[:, g0:, :qw],
                                      in_=o_all[:, g0:, :qw])
                else:
                    nc.sync.dma_start(out=dst[:, :, :qw],
                                      in_=o_all[:, :, :qw])

            for i, blk in enumerate(blocks):
                emit_scores(blk)
                if i > 0:
                    emit_ctx_out(i - 1, blocks[i - 1])
            emit_ctx_out(len(blocks) - 1, blocks[-1])
            attp.release()
            xtp.release()

    nc.compile()
    return nc


def _prepare(X, masks, Wq, Wk, Wv, Wo):
    import ml_dtypes
    BF = ml_dtypes.bfloat16
    F8 = ml_dtypes.float8_e4m3

    X = np.asarray(X, dtype=np.float32)
    masks = np.asarray(masks)
    Wq = np.asarray(Wq, dtype=np.float32)
    Wk = np.asarray(Wk, dtype=np.float32)
    Wv = np.asarray(Wv, dtype=np.float32)
    Wo = np.asarray(Wo, dtype=np.float32)

    idxs = [np.where(masks[b] != 0)[0] for b in range(B)]
    # 256-multiples so fp8 DoubleRow k-tile pairs align for both batches
    LQs = [max(256, int(-(-len(ix) // 256) * 256)) for ix in idxs]
    LT = sum(LQs)
    offs = [0, LQs[0]]
    QBK = 512
    NBLK = -(-LT // QBK)
    LT_pad = NBLK * QBK
    n_dblk = D // 128

    # compacted, transposed X: columns = valid tokens (zero-padded)
    XTc = np.zeros((D, LT_pad), dtype=np.float32)
    for b in range(B):
        XTc[:, offs[b]:offs[b] + len(idxs[b])] = X[b].T[:, idxs[b]]
    # DMA-friendly: [NBLK, 128, n_dblk*QBK], per-partition contiguous
    XTa = np.ascontiguousarray(
        XTc.reshape(n_dblk, 128, NBLK, QBK).transpose(2, 1, 0, 3)
        .reshape(NBLK, 128, n_dblk * QBK)).astype(BF)

    caus = (np.arange(896)[None, :] - 384 >= np.arange(128)[:, None])

    nc = _build_program(LQs)

    def warr(wT):  # [D, JW] -> [128, n_dblk*JW] per-partition contiguous
        return np.ascontiguousarray(
            wT.reshape(n_dblk, 128, JW).transpose(1, 0, 2)
            .reshape(128, n_dblk * JW)).astype(BF)

    in_maps = []
    for c in range(NCORES):
        jsl = slice(c * JW, (c + 1) * JW)
        ind = np.zeros((JW, NH), dtype=np.float32)
        for h in range(NH):
            ind[h * DH:(h + 1) * DH, h] = 1.0
        in_maps.append({
            "XT": XTa,
            "WALL": np.ascontiguousarray(np.concatenate(
                [warr(Wq[jsl, :].T), warr(Wk[jsl, :].T),
                 warr(Wv[jsl, :].T)], axis=1)),
            "WOT": np.ascontiguousarray(Wo[:, jsl].T).astype(BF),
            "CAUS": caus.astype(BF),
            "IND": ind.astype(BF),
            "INDT": np.ascontiguousarray(ind.T),
        })

    return nc, in_maps, (idxs, LQs, LT, offs)


def _unshard(results, meta):
    idxs, LQs, LT, offs = meta
    n_dblk = D // 128
    blocks = []
    for b in range(B):
        q0 = 0
        while q0 < LQs[b]:
            qw = min(QB, LQs[b] - q0)
            blocks.append((b, q0, qw))
            q0 += qw

    partial = np.zeros((D, LT), dtype=np.float64)
    for c in range(NCORES):
        # OUTT[i, p, g*QB + w] = out[g*128 + p, ob + q0 + w] for block i
        ot = results[c]["OUTT"].astype(np.float64).reshape(
            len(blocks), 128, n_dblk, QB)
        for i, (b, q0, qw) in enumerate(blocks):
            cols = slice(offs[b] + q0, offs[b] + q0 + qw)
            partial[:, cols] += ot[i, :, :, :qw].transpose(1, 0, 2).reshape(
                D, qw)
    partial = partial.T  # [LT, D]

    out = np.zeros((B, S, D), dtype=np.float32)
    for b in range(B):
        out[b, idxs[b], :] = partial[offs[b]:offs[b] + len(idxs[b]), :].astype(
            np.float32)
    return out


def kernel(X, masks, Wq, Wk, Wv, Wo):
    from concourse.bass_utils import run_bass_kernel_spmd

    nc, in_maps, meta = _prepare(X, masks, Wq, Wk, Wv, Wo)
    res = run_bass_kernel_spmd(nc, in_maps, list(range(NCORES)))
    return _unshard(res.results, meta)


def profile_run(inputs, tmpdir=None):
    """Used by test.py: same program, run with NTFF tracing enabled."""
    from concourse.bass_utils import run_bass_kernel_spmd

    nc, in_maps, meta = _prepare(**inputs)
    res = run_bass_kernel_spmd(nc, in_maps, list(range(NCORES)), trace=True,
                               tmpdir=tmpdir)
    res.output = _unshard(res.results, meta)
    return res



# revision 58
# speedup vs baseline: 1.2203x; 1.0625x over previous
"""Sparse (relu-cosine, causal+padding-masked) attention on 8 TRN2 NeuronCores.

Contract: kernel(**inputs) takes the full unsharded inputs and returns the
full [B, S, D] output. Internally:
  - host: compact each batch's tokens to the mask-valid ones (queries and
    keys share the same validity mask, so causal structure stays exactly
    lower-triangular in compacted space and all masking disappears),
    transpose X, slice per-head-pair weights, pad to tile multiples.
  - device (SPMD, 8 cores, 2 heads per core): QKV projections, cosine
    normalization folded into the relu scale (1/||k||) and a per-query
    broadcast tile (1/||q||), relu(QK^T) with triangular masks only on
    diagonal tiles, context accumulation (col-tiled pairs), and a partial
    output projection (transposed layout) through this core's 128 columns
    of Wo.
  - host: sum the 8 partial outputs, scatter rows back to the full
    [B, S, D] layout (masked query rows are exactly zero).

Matmul operands are bf16; every accumulation (PSUM) is fp32 and the
norm scales (1/||q||, 1/||k||) are computed from the fp32 sums, so the
cosine normalization is exact for the bf16-rounded Q/K. Attention is
software-pipelined per q-block: scores of block i+1 are issued to the PE
before the context matmuls of block i, so the PE never waits on relu.
"""

import numpy as np

B, S, D, H = 2, 2048, 1024, 16
DH = D // H
NCORES = 8
HEADS_PER_CORE = H // NCORES  # 2
NH = HEADS_PER_CORE
JW = HEADS_PER_CORE * DH  # 128, per-core head-dim slice width
QB = 512  # query block width (one fp32 PSUM bank)
KT = 128  # key tile (partition dim)


def _build_program(LQs, n_dblk=D // 128):
    import concourse.bass as bass
    import concourse.mybir as mybir
    import concourse.tile as tile
    from concourse import bacc
    from concourse.bass import ts
    from concourse.masks import make_identity

    F32 = mybir.dt.float32
    F32R = mybir.dt.float32r
    F16 = mybir.dt.float16
    BF16 = mybir.dt.bfloat16
    F8E4 = mybir.dt.float8e4
    DROW = mybir.MatmulPerfMode.DoubleRow
    AF = mybir.ActivationFunctionType
    MULT = mybir.AluOpType.mult
    MAX = mybir.AluOpType.max

    LT = sum(LQs)
    offs = [0, LQs[0]]
    n_ttiles = LT // 128
    NBLK = -(-LT // QB)
    LT_pad = NBLK * QB

    nc = bacc.Bacc("TRN2", target_bir_lowering=False, debug=False,
                   num_devices=NCORES)

    # all big DRAM tensors are host-prearranged so every DMA is one
    # instruction with large per-partition-contiguous descriptors (one
    # dma_start already spreads over all 16 SDMA engines; many small
    # dma_starts just pay ~0.7us DIRECT2D issue cost each on the sync
    # sequencer).
    XT = nc.dram_tensor("XT", [NBLK, 128, n_dblk * QB], BF16,
                        kind="ExternalInput").ap()
    # q/k/v weights batched into one DMA: [128, 3, n_dblk*JW]
    WALL = nc.dram_tensor("WALL", [128, 3 * n_dblk * JW], BF16,
                          kind="ExternalInput").ap()
    WOT = nc.dram_tensor("WOT", [JW, D], BF16, kind="ExternalInput").ap()
    # causal diag-tile masks: CAUS[:, 384-off : 384-off+qbw], off = kt0-q0
    CAUS = nc.dram_tensor("CAUS", [128, 896], BF16, kind="ExternalInput").ap()
    # IND[j, h] = 1 if j // DH == h ; INDT is its transpose
    IND = nc.dram_tensor("IND", [JW, NH], BF16, kind="ExternalInput").ap()
    INDT = nc.dram_tensor("INDT", [NH, JW], F32R, kind="ExternalInput").ap()
    # transposed output, per-q-block contiguous; host reassembles
    n_qblocks = sum(-(-lq // QB) for lq in LQs)
    OUTT = nc.dram_tensor("OUTT", [n_qblocks, 128, n_dblk * QB], F16,
                          kind="ExternalOutput").ap()

    EPS = 1e-12

    def col_blocks(width, bw=QB):
        blocks = []
        c = 0
        while c < width:
            w = min(bw, width - c)
            blocks.append((c, w))
            c += w
        return blocks

    with tile.TileContext(nc) as tc:
        with (
            tc.tile_pool(name="consts", bufs=1) as consts,
            tc.tile_pool(name="proj", bufs=1) as projp,
            tc.tile_pool(name="work", bufs=3) as work,
            tc.tile_pool(name="outp", bufs=2) as outp,
            tc.tile_pool(name="ps_mm", bufs=1, space="PSUM") as ps_mm,
            tc.tile_pool(name="ps_scp", bufs=3, space="PSUM") as ps_scp,
            tc.tile_pool(name="ps_ctxp", bufs=1, space="PSUM") as ps_ctxp,
        ):
            # ---- weights first (first projection matmul needs them); the
            # sync sequencer issues [weights, xt blocks] only, in that order:
            # every DIRECT2D issue ahead of the first xt block delays the PE
            # ramp, so everything else goes through the scalar HWDGE queue.
            wall = consts.tile([128, 3, n_dblk, JW], BF16)
            nc.sync.dma_start(
                out=wall,
                in_=WALL.rearrange("p (s k j) -> p s k j", s=3, k=n_dblk))
            wqt, wkt, wvt = wall[:, 0], wall[:, 1], wall[:, 2]

            xtp = tc.alloc_tile_pool(name="xt", bufs=1)
            # ---- X^T (all d-blocks resident), one DMA per 512-col block ----
            # block-major layout: the first projection block lands first, and
            # each DMA is 1MB of 8KB-contiguous per-partition runs.
            xt = xtp.tile([128, NBLK, n_dblk, QB], BF16)
            for b in range(NBLK):
                nc.sync.dma_start(
                    out=xt[:, b],
                    in_=XT[b].rearrange("p (k w) -> p k w", k=n_dblk))

            # ---- remaining constants (scalar HWDGE queue) ------------------
            caus = consts.tile([128, 896], BF16)
            nc.scalar.dma_start(out=caus, in_=CAUS[:, :])
            wot = consts.tile([JW, D], BF16)
            nc.scalar.dma_start(out=wot, in_=WOT[:, :])
            ind = consts.tile([JW, NH], BF16)
            nc.scalar.dma_start(out=ind, in_=IND[:, :])
            indt = consts.tile([NH, JW], F32R)
            nc.scalar.dma_start(out=indt, in_=INDT[:, :])
            eps128 = consts.tile([128, 1], F32)
            nc.vector.memset(eps128, EPS)
            ident = consts.tile([128, 128], BF16)
            make_identity(nc, ident)

            # ---- projections ------------------------------------------------
            # qt/kt/vt in bf16; squares for the norms are taken from the f32
            # PSUM so the scales stay exact for the rounded Q/K.
            qt = projp.tile([JW, LT], BF16)
            kt_ = projp.tile([JW, LT], BF16)
            vt = projp.tile([JW, LT], BF16)
            qsq = projp.tile([JW, LT], BF16)
            ksq = projp.tile([JW, LT], BF16)
            cp_i = 0
            for bi, (c0, w) in enumerate(col_blocks(LT)):
                for dst, wmat, sq in ((qt, wqt, qsq), (kt_, wkt, ksq),
                                      (vt, wvt, None)):
                    ps = ps_scp.tile([JW, QB], F32, tag="sc", name="ps_proj")
                    for k in range(n_dblk):
                        nc.tensor.matmul(
                            ps[:, :w], wmat[:, k, :], xt[:, bi, k, :w],
                            start=(k == 0), stop=(k == n_dblk - 1),
                        )
                    if cp_i % 2 == 0:
                        nc.vector.tensor_copy(dst[:, c0:c0 + w], ps[:, :w])
                    else:
                        nc.scalar.activation(out=dst[:, c0:c0 + w],
                                             in_=ps[:, :w], func=AF.Copy)
                    cp_i += 1
                    if sq is not None:
                        # gpsimd is otherwise idle and these are SBUF->SBUF
                        nc.gpsimd.tensor_mul(sq[:, c0:c0 + w],
                                             dst[:, c0:c0 + w],
                                             dst[:, c0:c0 + w])

            # ---- normalize queries in place: qt *= 1/|q| --------------------
            # (hoisted out of the attention loop; scores are linear in q and
            # qsb partition p carries head(p)'s scale, matching qt's layout)
            for c0, w in col_blocks(LT):
                ps_ss = ps_mm.tile([NH, QB], F32, tag="mm", name="ps_qsum")
                nc.tensor.matmul(ps_ss[:, :w], ind[:, :], qsq[:, c0:c0 + w],
                                 start=True, stop=True)
                ssq = work.tile([NH, QB], F32R, tag="ssq")
                nc.scalar.activation(out=ssq[:, :w], in_=ps_ss[:, :w],
                                     func=AF.Sqrt, bias=eps128[:NH, :],
                                     scale=1.0)
                ps_qsb = ps_mm.tile([128, QB], F32, tag="mm", name="ps_qsb")
                nc.tensor.matmul(ps_qsb[:, :w], indt[:, :], ssq[:, :w],
                                 start=True, stop=True)
                qsb = work.tile([128, QB], F32, tag="qsb")
                nc.vector.reciprocal_approx_fast(out=qsb[:, :w],
                                                 in_=ps_qsb[:, :w])
                nc.vector.tensor_mul(qt[:, c0:c0 + w], qt[:, c0:c0 + w],
                                     qsb[:, :w])

            # ---- kscale[t, h] = rsqrt(sum_j ksq[j, t] over head h) ----------
            ksum_ps = ps_ctxp.tile([128, n_ttiles * NH], F32, tag="ctx_ps",
                                   name="ksum_ps")
            for tt in range(n_ttiles):
                nc.tensor.matmul(ksum_ps[:, tt * NH:(tt + 1) * NH],
                                 ksq[:, ts(tt, 128)], ind[:, :],
                                 start=True, stop=True, skip_group_check=True)
            ksc = projp.tile([128, n_ttiles, NH], F32)
            nc.scalar.activation(out=ksc[:, :, :].rearrange("p a b -> p (a b)"),
                                 in_=ksum_ps[:, :], func=AF.Sqrt,
                                 bias=eps128[:, :], scale=1.0)
            nc.vector.reciprocal_approx_fast(
                out=ksc[:, :, :].rearrange("p a b -> p (a b)"),
                in_=ksc[:, :, :].rearrange("p a b -> p (a b)"))

            # ---- V natural via PE transpose --------------------------------
            vn = projp.tile([128, n_ttiles, JW], BF16)
            for tt in range(n_ttiles):
                ps = ps_scp.tile([128, 128], BF16, tag="sc", name="ps_vtr")
                nc.tensor.transpose(ps[:, :], vt[:, ts(tt, 128)], ident)
                for h in range(NH):
                    if tt % 2 == 0:
                        nc.vector.tensor_scalar_mul(
                            out=vn[:, tt, ts(h, DH)], in0=ps[:, ts(h, DH)],
                            scalar1=ksc[:, tt, h:h + 1])
                    else:
                        nc.scalar.activation(
                            out=vn[:, tt, ts(h, DH)], in_=ps[:, ts(h, DH)],
                            func=AF.Copy, scale=ksc[:, tt, h:h + 1])

            # keep xt resident: releasing it here would make the att pool
            # reuse its SBUF zone, serializing attention start behind the
            # last projection matmul. Both fit in SBUF at bf16 sizes.
            max_nkt = max(LQs) // KT
            att_bufs = 3 if max_nkt <= 10 else (2 if max_nkt <= 14 else 1)
            attp = tc.alloc_tile_pool(name="att", bufs=att_bufs)



            # ---- attention, software-pipelined over q-blocks ----------------
            blocks = []
            for b in range(B):
                for q0, qw in col_blocks(LQs[b]):
                    blocks.append((b, q0, qw))
            ctx_sbs = {
                b: attp.tile([JW, LQs[b]], BF16, tag=f"ctx_{b}", bufs=1,
                             name=f"ctx_sb{b}")
                for b in range(B)
            }

            state = {}

            def emit_scores(blk):
                b, q0, qw = blk
                ob = offs[b]
                lq = LQs[b]
                n_kt = min((q0 + qw + KT - 1) // KT, lq // KT)
                att_sb = attp.tile([128, max_nkt * NH, QB], BF16,
                                   tag="att_sb", name="att_sb")
                offs_ki = []
                diag_i = 0
                relu_i = 0
                for ki in range(n_kt):
                    k0 = ki * KT
                    # columns < off are fully masked by causality; skip them
                    off = max(0, k0 - q0)
                    offs_ki.append(off)
                    w = qw - off
                    diag = k0 > q0 - KT
                    sc_ps = ps_scp.tile([128, NH, QB], F32, tag="sc",
                                        name="sc_ps")
                    for h in range(NH):
                        nc.tensor.matmul(
                            sc_ps[:, h, off:qw],
                            kt_[ts(h, DH), ob + k0:ob + k0 + KT],
                            qt[ts(h, DH), ob + q0 + off:ob + q0 + qw],
                            start=True, stop=True,
                        )
                    # att = relu(s) for both heads in one op (k-norm scale
                    # lives in V); diagonal tiles fuse the triangular mask:
                    # (s max 0) * caus, with caus broadcast over the head dim
                    sl = ki * NH
                    slot = att_sb[:, sl:sl + NH, off:qw]
                    if diag:
                        cs = caus[:, 384:384 + w]
                        cs2 = bass.AP(tensor=cs.tensor, offset=cs.offset,
                                      ap=[cs.ap[0], [0, NH], cs.ap[1]])
                        if diag_i % 2 == 0:
                            nc.vector.scalar_tensor_tensor(
                                out=slot, in0=sc_ps[:, :, off:qw], scalar=0.0,
                                in1=cs2, op0=MAX, op1=MULT)
                        else:
                            nc.scalar.activation(
                                out=slot, in_=sc_ps[:, :, off:qw],
                                func=AF.Relu)
                            nc.vector.tensor_mul(slot, slot, cs2)
                        diag_i += 1
                    else:
                        # DVE max is ~1.5x faster per element than the ACT
                        # relu here; scalar gets the out-proj copies instead
                        if relu_i % 4 == 3:
                            nc.scalar.activation(
                                out=slot, in_=sc_ps[:, :, off:qw],
                                func=AF.Relu)
                        else:
                            nc.vector.tensor_scalar_max(
                                out=slot, in0=sc_ps[:, :, off:qw],
                                scalar1=0.0)
                        relu_i += 1
                state[blk] = (att_sb, n_kt, offs_ki)

            def emit_ctx_out(blk_i, blk):
                b, q0, qw = blk
                ob = offs[b]
                ctx_sb = ctx_sbs[b]
                att_sb, n_kt, offs_ki = state.pop(blk)
                # col-tiled pair: both heads accumulate in one PSUM bank
                ctx_ps = ps_ctxp.tile([128, QB], F32, tag="ctx_ps",
                                      name="ctx_ps")
                assert offs_ki[0] == 0  # first tile always starts the bank
                for ki in range(n_kt):
                    gtt = (ob + ki * KT) // KT
                    off = offs_ki[ki]
                    for h in range(NH):
                        nc.tensor.matmul(
                            ctx_ps[ts(h, DH), off:qw],
                            vn[:, gtt, ts(h, DH)],
                            att_sb[:, ki * NH + h, off:qw],
                            start=(ki == 0), stop=(ki == n_kt - 1),
                            tile_position=(0, h * DH),
                            skip_group_check=True,
                        )
                # 1/|q| already folded into the queries; plain copy, on the
                # scalar engine to keep vector off this block-boundary chain
                nc.scalar.activation(out=ctx_sb[:, q0:q0 + qw],
                                     in_=ctx_ps[:, :qw], func=AF.Copy)

                # output projection (transposed layout), this q-block only;
                # dblk pairs share one 2-bank PSUM tile; all 8 dout chunks
                # gather into one SBUF tile and ship in a single DMA.
                o_all = outp.tile([128, n_dblk, QB], F16, tag="o_sb")
                dst = OUTT[blk_i].rearrange("p (g w) -> p g w", g=n_dblk)
                last = blk_i == n_qblocks - 1
                for dp in range(n_dblk // 2):
                    ps = ps_scp.tile([128, 2, QB], F32, tag="sc",
                                     name="ps_out")
                    for two in range(2):
                        nc.tensor.matmul(ps[:, two, :qw],
                                         wot[:, ts(dp * 2 + two, 128)],
                                         ctx_sb[:, q0:q0 + qw],
                                         start=True, stop=True)
                    if dp % 2 == 0:
                        nc.vector.tensor_copy(
                            o_all[:, dp * 2:dp * 2 + 2, :qw], ps[:, :, :qw])
                    else:
                        nc.scalar.activation(
                            out=o_all[:, dp * 2:dp * 2 + 2, :qw],
                            in_=ps[:, :, :qw], func=AF.Copy)
                    if last and dp == n_dblk // 2 - 2:
                        # overlap most of the final store with the last
                        # chunk's compute to shorten the drain tail
                        nc.sync.dma_start(out=dst[:, :dp * 2 + 2, :qw],
                                          in_=o_all[:, :dp * 2 + 2, :qw])
                if last:
                    g0 = n_dblk - 2
                    nc.sync.dma_start(out=dst[:, g0:, :qw],
                                      in_=o_all[:, g0:, :qw])
                else:
                    nc.sync.dma_start(out=dst[:, :, :qw],
                                      in_=o_all[:, :, :qw])

            for i, blk in enumerate(blocks):
                emit_scores(blk)
                if i > 0:
                    emit_ctx_out(i - 1, blocks[i - 1])
            emit_ctx_out(len(blocks) - 1, blocks[-1])
            attp.release()
            xtp.release()

    nc.compile()
    return nc


def _prepare(X, masks, Wq, Wk, Wv, Wo):
    import ml_dtypes
    BF = ml_dtypes.bfloat16
    F8 = ml_dtypes.float8_e4m3

    X = np.asarray(X, dtype=np.float32)
    masks = np.asarray(masks)
    Wq = np.asarray(Wq, dtype=np.float32)
    Wk = np.asarray(Wk, dtype=np.float32)
    Wv = np.asarray(Wv, dtype=np.float32)
    Wo = np.asarray(Wo, dtype=np.float32)

    idxs = [np.where(masks[b] != 0)[0] for b in range(B)]
    # 256-multiples so fp8 DoubleRow k-tile pairs align for both batches
    LQs = [max(256, int(-(-len(ix) // 256) * 256)) for ix in idxs]
    LT = sum(LQs)
    offs = [0, LQs[0]]
    QBK = 512
    NBLK = -(-LT // QBK)
    LT_pad = NBLK * QBK
    n_dblk = D // 128

    # compacted, transposed X: columns = valid tokens (zero-padded)
    XTc = np.zeros((D, LT_pad), dtype=np.float32)
    for b in range(B):
        XTc[:, offs[b]:offs[b] + len(idxs[b])] = X[b].T[:, idxs[b]]
    # DMA-friendly: [NBLK, 128, n_dblk*QBK], per-partition contiguous
    XTa = np.ascontiguousarray(
        XTc.reshape(n_dblk, 128, NBLK, QBK).transpose(2, 1, 0, 3)
        .reshape(NBLK, 128, n_dblk * QBK)).astype(BF)

    caus = (np.arange(896)[None, :] - 384 >= np.arange(128)[:, None])

    nc = _build_program(LQs)

    def warr(wT):  # [D, JW] -> [128, n_dblk*JW] per-partition contiguous
        return np.ascontiguousarray(
            wT.reshape(n_dblk, 128, JW).transpose(1, 0, 2)
            .reshape(128, n_dblk * JW)).astype(BF)

    in_maps = []
    for c in range(NCORES):
        jsl = slice(c * JW, (c + 1) * JW)
        ind = np.zeros((JW, NH), dtype=np.float32)
        for h in range(NH):
            ind[h * DH:(h + 1) * DH, h] = 1.0
        in_maps.append({
            "XT": XTa,
            "WALL": np.ascontiguousarray(np.concatenate(
                [warr(Wq[jsl, :].T), warr(Wk[jsl, :].T),
                 warr(Wv[jsl, :].T)], axis=1)),
            "WOT": np.ascontiguousarray(Wo[:, jsl].T).astype(BF),
            "CAUS": caus.astype(BF),
            "IND": ind.astype(BF),
            "INDT": np.ascontiguousarray(ind.T),
        })

    return nc, in_maps, (idxs, LQs, LT, offs)


def _unshard(results, meta):
    idxs, LQs, LT, offs = meta
    n_dblk = D // 128
    blocks = []
    for b in range(B):
        q0 = 0
        while q0 < LQs[b]:
            qw = min(QB, LQs[b] - q0)
            blocks.append((b, q0, qw))
            q0 += qw

    partial = np.zeros((D, LT), dtype=np.float64)
    for c in range(NCORES):
        # OUTT[i, p, g*QB + w] = out[g*128 + p, ob + q0 + w] for block i
        ot = results[c]["OUTT"].astype(np.float64).reshape(
            len(blocks), 128, n_dblk, QB)
        for i, (b, q0, qw) in enumerate(blocks):
            cols = slice(offs[b] + q0, offs[b] + q0 + qw)
            partial[:, cols] += ot[i, :, :, :qw].transpose(1, 0, 2).reshape(
                D, qw)
    partial = partial.T  # [LT, D]

    out = np.zeros((B, S, D), dtype=np.float32)
    for b in range(B):
        out[b, idxs[b], :] = partial[offs[b]:offs[b] + len(idxs[b]), :].astype(
            np.float32)
    return out


def kernel(X, masks, Wq, Wk, Wv, Wo):
    from concourse.bass_utils import run_bass_kernel_spmd

    nc, in_maps, meta = _prepare(X, masks, Wq, Wk, Wv, Wo)
    res = run_bass_kernel_spmd(nc, in_maps, list(range(NCORES)))
    return _unshard(res.results, meta)


def profile_run(inputs, tmpdir=None):
    """Used by test.py: same program, run with NTFF tracing enabled."""
    from concourse.bass_utils import run_bass_kernel_spmd

    nc, in_maps, meta = _prepare(**inputs)
    res = run_bass_kernel_spmd(nc, in_maps, list(range(NCORES)), trace=True,
                               tmpdir=tmpdir)
    res.output = _unshard(res.results, meta)
    return res



# revision 59
# speedup vs baseline: 1.2576x; 1.0306x over previous
"""Sparse (relu-cosine, causal+padding-masked) attention on 8 TRN2 NeuronCores.

Contract: kernel(**inputs) takes the full unsharded inputs and returns the
full [B, S, D] output. Internally:
  - host: compact each batch's tokens to the mask-valid ones (queries and
    keys share the same validity mask, so causal structure stays exactly
    lower-triangular in compacted space and all masking disappears),
    transpose X, slice per-head-pair weights, pad to tile multiples.
  - device (SPMD, 8 cores, 2 heads per core): QKV projections, cosine
    normalization folded into the relu scale (1/||k||) and a per-query
    broadcast tile (1/||q||), relu(QK^T) with triangular masks only on
    diagonal tiles, context accumulation (col-tiled pairs), and a partial
    output projection (transposed layout) through this core's 128 columns
    of Wo.
  - host: sum the 8 partial outputs, scatter rows back to the full
    [B, S, D] layout (masked query rows are exactly zero).

Matmul operands are bf16; every accumulation (PSUM) is fp32 and the
norm scales (1/||q||, 1/||k||) are computed from the fp32 sums, so the
cosine normalization is exact for the bf16-rounded Q/K. Attention is
software-pipelined per q-block: scores of block i+1 are issued to the PE
before the context matmuls of block i, so the PE never waits on relu.
"""

import numpy as np

B, S, D, H = 2, 2048, 1024, 16
DH = D // H
NCORES = 8
HEADS_PER_CORE = H // NCORES  # 2
NH = HEADS_PER_CORE
JW = HEADS_PER_CORE * DH  # 128, per-core head-dim slice width
QB = 512  # query block width (one fp32 PSUM bank)
KT = 128  # key tile (partition dim)


def _build_program(LQs, n_dblk=D // 128):
    import concourse.bass as bass
    import concourse.mybir as mybir
    import concourse.tile as tile
    from concourse import bacc
    from concourse.bass import ts
    from concourse.masks import make_identity

    F32 = mybir.dt.float32
    F32R = mybir.dt.float32r
    F16 = mybir.dt.float16
    BF16 = mybir.dt.bfloat16
    F8E4 = mybir.dt.float8e4
    DROW = mybir.MatmulPerfMode.DoubleRow
    AF = mybir.ActivationFunctionType
    MULT = mybir.AluOpType.mult
    MAX = mybir.AluOpType.max

    LT = sum(LQs)
    offs = [0, LQs[0]]
    n_ttiles = LT // 128
    NBLK = -(-LT // QB)
    LT_pad = NBLK * QB

    nc = bacc.Bacc("TRN2", target_bir_lowering=False, debug=False,
                   num_devices=NCORES)

    # all big DRAM tensors are host-prearranged so every DMA is one
    # instruction with large per-partition-contiguous descriptors (one
    # dma_start already spreads over all 16 SDMA engines; many small
    # dma_starts just pay ~0.7us DIRECT2D issue cost each on the sync
    # sequencer).
    XT = nc.dram_tensor("XT", [NBLK, 128, n_dblk * QB], BF16,
                        kind="ExternalInput").ap()
    # q/k/v weights batched into one DMA: [128, 3, n_dblk*JW]
    WALL = nc.dram_tensor("WALL", [128, 3 * n_dblk * JW], BF16,
                          kind="ExternalInput").ap()
    WOT = nc.dram_tensor("WOT", [JW, D], BF16, kind="ExternalInput").ap()
    # causal diag-tile masks: CAUS[:, 384-off : 384-off+qbw], off = kt0-q0
    CAUS = nc.dram_tensor("CAUS", [128, 896], BF16, kind="ExternalInput").ap()
    # IND[j, h] = 1 if j // DH == h ; INDT is its transpose
    IND = nc.dram_tensor("IND", [JW, NH], BF16, kind="ExternalInput").ap()
    INDT = nc.dram_tensor("INDT", [NH, JW], F32R, kind="ExternalInput").ap()
    # transposed output, per-q-block contiguous; host reassembles
    n_qblocks = sum(-(-lq // QB) for lq in LQs)
    OUTT = nc.dram_tensor("OUTT", [n_qblocks, 128, n_dblk * QB], F16,
                          kind="ExternalOutput").ap()

    EPS = 1e-12

    def col_blocks(width, bw=QB):
        blocks = []
        c = 0
        while c < width:
            w = min(bw, width - c)
            blocks.append((c, w))
            c += w
        return blocks

    with tile.TileContext(nc) as tc:
        with (
            tc.tile_pool(name="consts", bufs=1) as consts,
            tc.tile_pool(name="proj", bufs=1) as projp,
            tc.tile_pool(name="work", bufs=3) as work,
            tc.tile_pool(name="outp", bufs=2) as outp,
            tc.tile_pool(name="ps_mm", bufs=1, space="PSUM") as ps_mm,
            tc.tile_pool(name="ps_scp", bufs=3, space="PSUM") as ps_scp,
            tc.tile_pool(name="ps_ctxp", bufs=1, space="PSUM") as ps_ctxp,
        ):
            # ---- weights first (first projection matmul needs them); the
            # sync sequencer issues [weights, xt blocks] only, in that order:
            # every DIRECT2D issue ahead of the first xt block delays the PE
            # ramp, so everything else goes through the scalar HWDGE queue.
            wall = consts.tile([128, 3, n_dblk, JW], BF16)
            nc.sync.dma_start(
                out=wall,
                in_=WALL.rearrange("p (s k j) -> p s k j", s=3, k=n_dblk))
            wqt, wkt, wvt = wall[:, 0], wall[:, 1], wall[:, 2]

            xtp = tc.alloc_tile_pool(name="xt", bufs=1)
            # ---- X^T (all d-blocks resident), one DMA per 512-col block ----
            # block-major layout: the first projection block lands first, and
            # each DMA is 1MB of 8KB-contiguous per-partition runs.
            xt = xtp.tile([128, NBLK, n_dblk, QB], BF16)
            for b in range(NBLK):
                nc.sync.dma_start(
                    out=xt[:, b],
                    in_=XT[b].rearrange("p (k w) -> p k w", k=n_dblk))

            # ---- remaining constants (scalar HWDGE queue) ------------------
            caus = consts.tile([128, 896], BF16)
            nc.scalar.dma_start(out=caus, in_=CAUS[:, :])
            wot = consts.tile([JW, D], BF16)
            nc.scalar.dma_start(out=wot, in_=WOT[:, :])
            ind = consts.tile([JW, NH], BF16)
            nc.scalar.dma_start(out=ind, in_=IND[:, :])
            indt = consts.tile([NH, JW], F32R)
            nc.scalar.dma_start(out=indt, in_=INDT[:, :])
            eps128 = consts.tile([128, 1], F32)
            nc.vector.memset(eps128, EPS)
            ident = consts.tile([128, 128], BF16)
            make_identity(nc, ident)

            # ---- projections ------------------------------------------------
            # qt/kt/vt in bf16; squares for the norms are taken from the f32
            # PSUM so the scales stay exact for the rounded Q/K.
            qt = projp.tile([JW, LT], BF16)
            kt_ = projp.tile([JW, LT], BF16)
            vt = projp.tile([JW, LT], BF16)
            qsq = projp.tile([JW, LT], BF16)
            ksq = projp.tile([JW, LT], BF16)
            cp_i = 0
            for bi, (c0, w) in enumerate(col_blocks(LT)):
                for dst, wmat, sq in ((qt, wqt, qsq), (kt_, wkt, ksq),
                                      (vt, wvt, None)):
                    ps = ps_scp.tile([JW, QB], F32, tag="sc", name="ps_proj")
                    for k in range(n_dblk):
                        nc.tensor.matmul(
                            ps[:, :w], wmat[:, k, :], xt[:, bi, k, :w],
                            start=(k == 0), stop=(k == n_dblk - 1),
                        )
                    if cp_i % 2 == 0:
                        nc.vector.tensor_copy(dst[:, c0:c0 + w], ps[:, :w])
                    else:
                        nc.scalar.activation(out=dst[:, c0:c0 + w],
                                             in_=ps[:, :w], func=AF.Copy)
                    cp_i += 1
                    if sq is not None:
                        # gpsimd is otherwise idle and these are SBUF->SBUF
                        nc.gpsimd.tensor_mul(sq[:, c0:c0 + w],
                                             dst[:, c0:c0 + w],
                                             dst[:, c0:c0 + w])

            # ---- normalize queries in place: qt *= 1/|q| --------------------
            # (hoisted out of the attention loop; scores are linear in q and
            # qsb partition p carries head(p)'s scale, matching qt's layout)
            for c0, w in col_blocks(LT):
                ps_ss = ps_mm.tile([NH, QB], F32, tag="mm", name="ps_qsum")
                nc.tensor.matmul(ps_ss[:, :w], ind[:, :], qsq[:, c0:c0 + w],
                                 start=True, stop=True)
                ssq = work.tile([NH, QB], F32R, tag="ssq")
                nc.scalar.activation(out=ssq[:, :w], in_=ps_ss[:, :w],
                                     func=AF.Sqrt, bias=eps128[:NH, :],
                                     scale=1.0)
                ps_qsb = ps_mm.tile([128, QB], F32, tag="mm", name="ps_qsb")
                nc.tensor.matmul(ps_qsb[:, :w], indt[:, :], ssq[:, :w],
                                 start=True, stop=True)
                qsb = work.tile([128, QB], F32, tag="qsb")
                nc.vector.reciprocal_approx_fast(out=qsb[:, :w],
                                                 in_=ps_qsb[:, :w])
                nc.vector.tensor_mul(qt[:, c0:c0 + w], qt[:, c0:c0 + w],
                                     qsb[:, :w])

            # ---- kscale[t, h] = rsqrt(sum_j ksq[j, t] over head h) ----------
            ksum_ps = ps_ctxp.tile([128, n_ttiles * NH], F32, tag="ctx_ps",
                                   name="ksum_ps")
            for tt in range(n_ttiles):
                nc.tensor.matmul(ksum_ps[:, tt * NH:(tt + 1) * NH],
                                 ksq[:, ts(tt, 128)], ind[:, :],
                                 start=True, stop=True, skip_group_check=True)
            ksc = projp.tile([128, n_ttiles, NH], F32)
            nc.scalar.activation(out=ksc[:, :, :].rearrange("p a b -> p (a b)"),
                                 in_=ksum_ps[:, :], func=AF.Sqrt,
                                 bias=eps128[:, :], scale=1.0)
            nc.vector.reciprocal_approx_fast(
                out=ksc[:, :, :].rearrange("p a b -> p (a b)"),
                in_=ksc[:, :, :].rearrange("p a b -> p (a b)"))

            # ---- V natural via PE transpose --------------------------------
            vn = projp.tile([128, n_ttiles, JW], BF16)
            for tt in range(n_ttiles):
                ps = ps_scp.tile([128, 128], BF16, tag="sc", name="ps_vtr")
                nc.tensor.transpose(ps[:, :], vt[:, ts(tt, 128)], ident)
                for h in range(NH):
                    if tt % 2 == 0:
                        nc.vector.tensor_scalar_mul(
                            out=vn[:, tt, ts(h, DH)], in0=ps[:, ts(h, DH)],
                            scalar1=ksc[:, tt, h:h + 1])
                    else:
                        nc.scalar.activation(
                            out=vn[:, tt, ts(h, DH)], in_=ps[:, ts(h, DH)],
                            func=AF.Copy, scale=ksc[:, tt, h:h + 1])

            # keep xt resident: releasing it here would make the att pool
            # reuse its SBUF zone, serializing attention start behind the
            # last projection matmul. Both fit in SBUF at bf16 sizes.
            max_nkt = max(LQs) // KT
            att_bufs = 3 if max_nkt <= 10 else (2 if max_nkt <= 14 else 1)
            attp = tc.alloc_tile_pool(name="att", bufs=att_bufs)



            # ---- attention, software-pipelined over q-blocks ----------------
            blocks = []
            for b in range(B):
                for q0, qw in col_blocks(LQs[b]):
                    blocks.append((b, q0, qw))
            ctx_sbs = {
                b: attp.tile([JW, LQs[b]], BF16, tag=f"ctx_{b}", bufs=1,
                             name=f"ctx_sb{b}")
                for b in range(B)
            }

            state = {}

            def emit_scores(blk):
                b, q0, qw = blk
                ob = offs[b]
                lq = LQs[b]
                n_kt = min((q0 + qw + KT - 1) // KT, lq // KT)
                att_sb = attp.tile([128, max_nkt * NH, QB], BF16,
                                   tag="att_sb", name="att_sb")
                offs_ki = []
                diag_i = 0
                relu_i = 0
                for ki in range(n_kt):
                    k0 = ki * KT
                    # columns < off are fully masked by causality; skip them
                    off = max(0, k0 - q0)
                    offs_ki.append(off)
                    w = qw - off
                    diag = k0 > q0 - KT
                    sc_ps = ps_scp.tile([128, NH, QB], F32, tag="sc",
                                        name="sc_ps")
                    for h in range(NH):
                        nc.tensor.matmul(
                            sc_ps[:, h, off:qw],
                            kt_[ts(h, DH), ob + k0:ob + k0 + KT],
                            qt[ts(h, DH), ob + q0 + off:ob + q0 + qw],
                            start=True, stop=True,
                        )
                    # att = relu(s) for both heads in one op (k-norm scale
                    # lives in V); diagonal tiles fuse the triangular mask:
                    # (s max 0) * caus, with caus broadcast over the head dim
                    sl = ki * NH
                    slot = att_sb[:, sl:sl + NH, off:qw]
                    if diag:
                        cs = caus[:, 384:384 + w]
                        cs2 = bass.AP(tensor=cs.tensor, offset=cs.offset,
                                      ap=[cs.ap[0], [0, NH], cs.ap[1]])
                        if diag_i % 2 == 0:
                            nc.vector.scalar_tensor_tensor(
                                out=slot, in0=sc_ps[:, :, off:qw], scalar=0.0,
                                in1=cs2, op0=MAX, op1=MULT)
                        else:
                            nc.scalar.activation(
                                out=slot, in_=sc_ps[:, :, off:qw],
                                func=AF.Relu)
                            nc.vector.tensor_mul(slot, slot, cs2)
                        diag_i += 1
                    else:
                        # DVE max is ~1.5x faster per element than the ACT
                        # relu, but vector also carries the diag masks, so
                        # split the plain relus evenly
                        if relu_i % 2 == 1:
                            nc.scalar.activation(
                                out=slot, in_=sc_ps[:, :, off:qw],
                                func=AF.Relu)
                        else:
                            nc.vector.tensor_scalar_max(
                                out=slot, in0=sc_ps[:, :, off:qw],
                                scalar1=0.0)
                        relu_i += 1
                state[blk] = (att_sb, n_kt, offs_ki)

            def emit_ctx_out(blk_i, blk):
                b, q0, qw = blk
                ob = offs[b]
                ctx_sb = ctx_sbs[b]
                att_sb, n_kt, offs_ki = state.pop(blk)
                # col-tiled pair: both heads accumulate in one PSUM bank
                ctx_ps = ps_ctxp.tile([128, QB], F32, tag="ctx_ps",
                                      name="ctx_ps")
                assert offs_ki[0] == 0  # first tile always starts the bank
                for ki in range(n_kt):
                    gtt = (ob + ki * KT) // KT
                    off = offs_ki[ki]
                    for h in range(NH):
                        nc.tensor.matmul(
                            ctx_ps[ts(h, DH), off:qw],
                            vn[:, gtt, ts(h, DH)],
                            att_sb[:, ki * NH + h, off:qw],
                            start=(ki == 0), stop=(ki == n_kt - 1),
                            tile_position=(0, h * DH),
                            skip_group_check=True,
                        )
                # 1/|q| already folded into the queries; plain copy, on the
                # scalar engine to keep vector off this block-boundary chain
                nc.scalar.activation(out=ctx_sb[:, q0:q0 + qw],
                                     in_=ctx_ps[:, :qw], func=AF.Copy)

                # output projection (transposed layout), this q-block only;
                # dblk pairs share one 2-bank PSUM tile; all 8 dout chunks
                # gather into one SBUF tile and ship in a single DMA.
                o_all = outp.tile([128, n_dblk, QB], F16, tag="o_sb")
                dst = OUTT[blk_i].rearrange("p (g w) -> p g w", g=n_dblk)
                last = blk_i == n_qblocks - 1
                for dp in range(n_dblk // 2):
                    ps = ps_scp.tile([128, 2, QB], F32, tag="sc",
                                     name="ps_out")
                    for two in range(2):
                        nc.tensor.matmul(ps[:, two, :qw],
                                         wot[:, ts(dp * 2 + two, 128)],
                                         ctx_sb[:, q0:q0 + qw],
                                         start=True, stop=True)
                    if dp % 2 == 0:
                        nc.vector.tensor_copy(
                            o_all[:, dp * 2:dp * 2 + 2, :qw], ps[:, :, :qw])
                    else:
                        nc.scalar.activation(
                            out=o_all[:, dp * 2:dp * 2 + 2, :qw],
                            in_=ps[:, :, :qw], func=AF.Copy)
                    if last and dp == n_dblk // 2 - 2:
                        # overlap most of the final store with the last
                        # chunk's compute to shorten the drain tail
                        nc.sync.dma_start(out=dst[:, :dp * 2 + 2, :qw],
                                          in_=o_all[:, :dp * 2 + 2, :qw])
                if last:
                    g0 = n_dblk - 2
                    nc.sync.dma_start(out=dst[:, g0:, :qw],
                                      in_=o_all[:, g0:, :qw])
                else:
                    nc.sync.dma_start(out=dst[:, :, :qw],
                                      in_=o_all[:, :, :qw])

            for i, blk in enumerate(blocks):
                emit_scores(blk)
                if i > 0:
                    emit_ctx_out(i - 1, blocks[i - 1])
            emit_ctx_out(len(blocks) - 1, blocks[-1])
            attp.release()
            xtp.release()

    nc.compile()
    return nc


def _prepare(X, masks, Wq, Wk, Wv, Wo):
    import ml_dtypes
    BF = ml_dtypes.bfloat16
    F8 = ml_dtypes.float8_e4m3

    X = np.asarray(X, dtype=np.float32)
    masks = np.asarray(masks)
    Wq = np.asarray(Wq, dtype=np.float32)
    Wk = np.asarray(Wk, dtype=np.float32)
    Wv = np.asarray(Wv, dtype=np.float32)
    Wo = np.asarray(Wo, dtype=np.float32)

    idxs = [np.where(masks[b] != 0)[0] for b in range(B)]
    # 256-multiples so fp8 DoubleRow k-tile pairs align for both batches
    LQs = [max(256, int(-(-len(ix) // 256) * 256)) for ix in idxs]
    LT = sum(LQs)
    offs = [0, LQs[0]]
    QBK = 512
    NBLK = -(-LT // QBK)
    LT_pad = NBLK * QBK
    n_dblk = D // 128

    # compacted, transposed X: columns = valid tokens (zero-padded)
    XTc = np.zeros((D, LT_pad), dtype=np.float32)
    for b in range(B):
        XTc[:, offs[b]:offs[b] + len(idxs[b])] = X[b].T[:, idxs[b]]
    # DMA-friendly: [NBLK, 128, n_dblk*QBK], per-partition contiguous
    XTa = np.ascontiguousarray(
        XTc.reshape(n_dblk, 128, NBLK, QBK).transpose(2, 1, 0, 3)
        .reshape(NBLK, 128, n_dblk * QBK)).astype(BF)

    caus = (np.arange(896)[None, :] - 384 >= np.arange(128)[:, None])

    nc = _build_program(LQs)

    def warr(wT):  # [D, JW] -> [128, n_dblk*JW] per-partition contiguous
        return np.ascontiguousarray(
            wT.reshape(n_dblk, 128, JW).transpose(1, 0, 2)
            .reshape(128, n_dblk * JW)).astype(BF)

    in_maps = []
    for c in range(NCORES):
        jsl = slice(c * JW, (c + 1) * JW)
        ind = np.zeros((JW, NH), dtype=np.float32)
        for h in range(NH):
            ind[h * DH:(h + 1) * DH, h] = 1.0
        in_maps.append({
            "XT": XTa,
            "WALL": np.ascontiguousarray(np.concatenate(
                [warr(Wq[jsl, :].T), warr(Wk[jsl, :].T),
                 warr(Wv[jsl, :].T)], axis=1)),
            "WOT": np.ascontiguousarray(Wo[:, jsl].T).astype(BF),
            "CAUS": caus.astype(BF),
            "IND": ind.astype(BF),
            "INDT": np.ascontiguousarray(ind.T),
        })

    return nc, in_maps, (idxs, LQs, LT, offs)


def _unshard(results, meta):
    idxs, LQs, LT, offs = meta
    n_dblk = D // 128
    blocks = []
    for b in range(B):
        q0 = 0
        while q0 < LQs[b]:
            qw = min(QB, LQs[b] - q0)
            blocks.append((b, q0, qw))
            q0 += qw

    partial = np.zeros((D, LT), dtype=np.float64)
    for c in range(NCORES):
        # OUTT[i, p, g*QB + w] = out[g*128 + p, ob + q0 + w] for block i
        ot = results[c]["OUTT"].astype(np.float64).reshape(
            len(blocks), 128, n_dblk, QB)
        for i, (b, q0, qw) in enumerate(blocks):
            cols = slice(offs[b] + q0, offs[b] + q0 + qw)
            partial[:, cols] += ot[i, :, :, :qw].transpose(1, 0, 2).reshape(
                D, qw)
    partial = partial.T  # [LT, D]

    out = np.zeros((B, S, D), dtype=np.float32)
    for b in range(B):
        out[b, idxs[b], :] = partial[offs[b]:offs[b] + len(idxs[b]), :].astype(
            np.float32)
    return out


def kernel(X, masks, Wq, Wk, Wv, Wo):
    from concourse.bass_utils import run_bass_kernel_spmd

    nc, in_maps, meta = _prepare(X, masks, Wq, Wk, Wv, Wo)
    res = run_bass_kernel_spmd(nc, in_maps, list(range(NCORES)))
    return _unshard(res.results, meta)


def profile_run(inputs, tmpdir=None):
    """Used by test.py: same program, run with NTFF tracing enabled."""
    from concourse.bass_utils import run_bass_kernel_spmd

    nc, in_maps, meta = _prepare(**inputs)
    res = run_bass_kernel_spmd(nc, in_maps, list(range(NCORES)), trace=True,
                               tmpdir=tmpdir)
    res.output = _unshard(res.results, meta)
    return res

